# revision 25
# baseline (speedup 1.0000x reference)
"""CapsuleLinear dynamic-routing kernel for TRN2 (8 NeuronCores, data-parallel over batch).

Math (reference):
    priors[n,j,i,k] = sum_l x[n,i,l] * w[j,k,l]          (never materialized: 151MB)
    3 routing iterations entirely in the L=8 compressed space:
      probs = softmax_j(logits)                          logits[n,i,j], init 0
      s[n,j,l]  = sum_i probs[n,j,i] * x[n,i,l]          (PE matmul, contraction over i)
      u[n,j,k]  = sum_l w[j,k,l] * s[n,j,l]              (DVE broadcast-mul + reduce)
      out       = squash_k(u)
      v[n,j,l]  = sum_k w[j,k,l] * out[n,j,k]            (DVE)
      logits   += sum_l x[n,i,l] * v[n,j,l]              (PE matmul, PSUM-resident accum)

Layout: i = 9*p + q  (p = SBUF partition 0..127, q = 0..8).
Softmax normalization is folded into x (xs = x * 1/d per i) so the wide (64-per-i)
probs tensor is never divided; sqrt is computed as exp(0.5*ln(.)) and Exp/Ln are
pinned to the single natural_log_exp ACT table set (no mid-kernel table reloads).
Matmul operands are bf16 (KFP32=1 env reverts to f32); all PSUM accumulation is f32.
"""

import os

import numpy as np

N, I, L, J, K = 32, 1152, 8, 64, 16
NCORES = 8
NPC = N // NCORES  # samples per core = 4
P = 128
Q = I // P  # 9
ITERS = 3
EPS = 1e-9

_cache = {}
LAST_RESULT = None


def _patch_act_tables():
    """Restrict Exp/Ln to the one table set containing both, so bacc's
    table-load pass never alternates sets (each reload costs ~2.7us)."""
    import concourse.hw_specs as hw_specs
    from concourse import mybir

    if getattr(hw_specs, "_capsule_patched", False):
        return
    orig = hw_specs.get_activation_tables

    def patched(arch):
        t = dict(orig(arch))
        AF = mybir.ActivationFunctionType
        both = "natural_log_exp_and_others"
        if both in t:
            for name in t:
                if name != both:
                    t[name] = t[name] - {AF.Exp, AF.Ln}
        return t

    hw_specs.get_activation_tables = patched
    hw_specs._capsule_patched = True


def _build():
    import concourse.bacc as bacc
    import concourse.tile as tile
    from concourse import mybir
    from concourse.masks import make_identity

    _patch_act_tables()

    f32 = mybir.dt.float32
    bf16 = mybir.dt.bfloat16
    mode = os.environ.get("KPREC", "split")  # split | split2 | bf16 | f32
    mmdt_a = bf16 if mode in ("split", "bf16") else f32   # (a)-path: e, xs, ones
    mmdt_b = bf16 if mode in ("split2", "bf16") else f32  # (b)-path: xT, vblk, identity
    AF = mybir.ActivationFunctionType

    nc = bacc.Bacc("TRN2", target_bir_lowering=False, debug=False, num_devices=NCORES)

    x_d = nc.dram_tensor("x", (NPC, I, L), f32, kind="ExternalInput")
    w_d = nc.dram_tensor("weight", (J, K, L), f32, kind="ExternalInput")
    o_d = nc.dram_tensor("out", (NPC, J, K), f32, kind="ExternalOutput")

    with tile.TileContext(nc) as tc:
        with tc.tile_pool(name="singles", bufs=1) as singles, \
             tc.tile_pool(name="work", bufs=2) as work:
            # ---- persistent SBUF tensors ----
            # xall[p, n, q, l] = x[n, 9p+q, l]; 288B contiguous runs; split in two
            # DMAs (different queues) to halve load latency
            xall = singles.tile([P, NPC, Q, L], f32)
            xsrc = x_d[:].rearrange("n (p q) l -> p n q l", p=P)
            nc.sync.dma_start(out=xall[:, 0:2], in_=xsrc[:, 0:2])
            nc.scalar.dma_start(out=xall[:, 2:4], in_=xsrc[:, 2:4])
            w_sb = singles.tile([J, K, L], f32)
            nc.sync.dma_start(out=w_sb, in_=w_d[:])

            id_t = singles.tile([P, P], mmdt_b)
            make_identity(nc, id_t)
            ones_t = singles.tile([P, P], mmdt_a)
            nc.gpsimd.memset(ones_t, 1.0)
            # block-diag v, padded: sample n occupies partitions 32n..32n+8
            # (compute-engine APs need 32-aligned base partitions)
            vblk = singles.tile([P, NPC * J], mmdt_b)
            nc.gpsimd.memset(vblk, 0.0)
            eps_t = singles.tile([J, 1], f32)
            nc.gpsimd.memset(eps_t, EPS)
            # padded v (pre-transpose): sample n in cols 32n..32n+8, zeros between
            vT_pad = singles.tile([J, NPC, 32], mmdt_b)
            nc.gpsimd.memset(vT_pad, 0.0)

            # warm the ACT ln/exp table set while DMAs run
            warm = singles.tile([1, 2], f32)
            nc.scalar.activation(warm[:, 0:1], eps_t[0:1, 0:1], AF.Ln)
            nc.scalar.activation(warm[:, 1:2], eps_t[0:1, 0:1], AF.Exp)

            # xqnl[p, q, n, l]: contiguous (n,l) runs per q -> single-free-dim
            # matmul operands (PE moving APs allow only one free dimension)
            xqnl = singles.tile([P, Q, NPC, L], f32)
            nc.vector.tensor_copy(xqnl, xall[:].transpose([0, 2, 1, 3]))
            # padded + cast copy: sample n in cols 32n..32n+8 per q, so one PE
            # transpose per q lands sample n at partitions 32n..32n+8
            xpad = singles.tile([P, Q, NPC, 32], mmdt_b)
            nc.gpsimd.memset(xpad, 0.0)
            nc.gpsimd.tensor_copy(xpad[:, :, :, 0:L], xqnl[:])

            # ---- setup: xT[32n+l, q, p] = x[n, 9p+q, l] via 9 PE transposes ----
            xT_sb = singles.tile([P, Q, P], mmdt_b)
            with tc.tile_pool(name="setup_ps", bufs=2, space="PSUM") as setup_ps:
                for q in range(Q):
                    xT_ps = setup_ps.tile([P, P], mmdt_b)
                    nc.tensor.transpose(
                        xT_ps, xpad[:, q].rearrange("p n l -> p (n l)"), id_t
                    )
                    if q % 2 == 0:
                        nc.scalar.copy(xT_sb[:, q, :], xT_ps)
                    else:
                        nc.vector.tensor_copy(xT_sb[:, q, :], xT_ps)

            with tc.tile_pool(name="logits_ps", bufs=1, space="PSUM") as logits_pool, \
                 tc.tile_pool(name="s2_ps", bufs=1, space="PSUM") as s2_pool, \
                 tc.tile_pool(name="vtr_ps", bufs=1, space="PSUM") as vtr_pool:
                logits = logits_pool.tile([P, Q, NPC, J], f32)  # PSUM-resident

                oc = None
                for t in range(ITERS):
                    # ---------- probs folded into xs = x/sum_j exp(logits) ----------
                    # slab-grouped (3 q's at a time) so (a) matmuls for early slabs
                    # overlap softmax work on later slabs
                    xs = work.tile([P, Q, NPC, L], mmdt_a, tag="xs")
                    if t == 0:
                        nc.vector.tensor_scalar_mul(xs, xqnl[:], 1.0 / J)
                        e_t = None
                    else:
                        e_t = work.tile([P, Q, NPC, J], mmdt_a, tag="e")
                        d_t = work.tile([P, Q, NPC], f32, tag="d")
                        r_t = work.tile([P, Q, NPC], f32, tag="r")
                        for g in range(3):
                            sl = slice(3 * g, 3 * g + 3)
                            nc.scalar.activation(e_t[:, sl], logits[:, sl], AF.Exp)
                            nc.vector.tensor_reduce(
                                d_t[:, sl],
                                e_t[:, sl],
                                axis=mybir.AxisListType.X,
                                op=mybir.AluOpType.add,
                            )
                            nc.vector.reciprocal(r_t[:, sl], d_t[:, sl])
                            nc.gpsimd.tensor_mul(
                                xs[:, sl],
                                xqnl[:, sl],
                                r_t[:, sl].unsqueeze(3).broadcast_to((P, 3, NPC, L)),
                            )

                    # ---------- (a): s2[(nh,j),(nh',l)] = sum_i e * xs ----------
                    # separate PSUM tiles per half: start=True clears a whole 2KB
                    # bank, so the two halves must not share one
                    s2a = s2_pool.tile([P, 2 * L], f32, tag="s2a")
                    s2b = s2_pool.tile([P, 2 * L], f32, tag="s2b")
                    for q in range(Q):
                        for h, s2 in enumerate((s2a, s2b)):
                            lhsT = (
                                ones_t[:]
                                if t == 0
                                else e_t[:, q, 2 * h : 2 * h + 2, :].rearrange(
                                    "p n j -> p (n j)"
                                )
                            )
                            nc.tensor.matmul(
                                s2[:],
                                lhsT,
                                xs[:, q, 2 * h : 2 * h + 2, :].rearrange(
                                    "p n l -> p (n l)"
                                ),
                                start=(q == 0),
                                stop=(q == Q - 1),
                            )
                    # extract diag blocks -> s_sb[j, n, l]
                    s_sb = work.tile([J, NPC, L], f32, tag="s_sb")
                    for n in range(NPC):
                        h, n2 = divmod(n, 2)
                        s2 = s2a if h == 0 else s2b
                        src = s2[64 * n2 : 64 * n2 + 64, 8 * n2 : 8 * n2 + 8]
                        if n % 2 == 0:
                            nc.scalar.copy(s_sb[:, n, :], src)
                        else:
                            nc.vector.tensor_copy(s_sb[:, n, :], src)

                    # ---------- u[j, n, k] = sum_l w[j,k,l] * s[j,n,l] ----------
                    pt = work.tile([J, NPC, K, L], f32, tag="pt")
                    nc.vector.tensor_mul(
                        pt,
                        w_sb[:].unsqueeze(1).broadcast_to((J, NPC, K, L)),
                        s_sb[:].unsqueeze(2).broadcast_to((J, NPC, K, L)),
                    )
                    u_t = work.tile([J, NPC, K], f32, tag="u")
                    nc.vector.reduce_sum(u_t, pt, axis=mybir.AxisListType.X)

                    # ---------- squash ----------
                    u2 = work.tile([J, NPC, K], f32, tag="u2")
                    nc.gpsimd.tensor_mul(u2, u_t, u_t)
                    sq = work.tile([J, NPC], f32, tag="sq")
                    nc.vector.reduce_sum(sq, u2, axis=mybir.AxisListType.X)
                    rt = work.tile([J, NPC], f32, tag="rt")  # sqrt(sq+eps)
                    nc.scalar.activation(rt, sq, AF.Ln, bias=eps_t[:])
                    nc.scalar.activation(rt, rt, AF.Exp, scale=0.5)
                    sp1 = work.tile([J, NPC], f32, tag="sp1")  # (1+sq)*sqrt(sq+eps)
                    nc.vector.tensor_scalar_add(sp1, sq, 1.0)
                    nc.vector.tensor_mul(sp1, sp1, rt)
                    nc.vector.reciprocal(sp1, sp1)
                    scl = work.tile([J, NPC], f32, tag="scl")
                    nc.vector.tensor_mul(scl, sq, sp1)
                    oc = work.tile([J, NPC, K], f32, tag="oc")
                    nc.vector.tensor_mul(
                        oc, u_t, scl[:].unsqueeze(2).broadcast_to((J, NPC, K))
                    )

                    if t == ITERS - 1:
                        break

                    # ---------- v[j, n, l] = sum_k w[j,k,l] * oc[j,n,k] ----------
                    qt = work.tile([J, NPC, L, K], f32, tag="qt")
                    nc.vector.tensor_mul(
                        qt,
                        w_sb[:].transpose([0, 2, 1]).unsqueeze(1).broadcast_to((J, NPC, L, K)),
                        oc[:].unsqueeze(2).broadcast_to((J, NPC, L, K)),
                    )
                    vT = work.tile([J, NPC, L], f32, tag="vT")
                    nc.vector.reduce_sum(vT, qt, axis=mybir.AxisListType.X)
                    # cast into the padded layout (sample n at cols 32n..32n+8)
                    nc.scalar.copy(vT_pad[:, :, 0:L], vT[:])

                    # one PE transpose: (64, 128) -> (128, 64); sample n lands at
                    # partitions 32n..32n+8 (32-aligned, so copies below are legal)
                    vtr = vtr_pool.tile([P, J], mmdt_b, tag="vtr")
                    nc.tensor.transpose(
                        vtr, vT_pad[:].rearrange("j n l -> j (n l)"), id_t[:J, :J]
                    )
                    for n in range(NPC):
                        src = vtr[32 * n : 32 * n + 8, :]
                        dst = vblk[32 * n : 32 * n + 8, 64 * n : 64 * n + 64]
                        if n % 2 == 0:
                            nc.scalar.copy(dst, src)
                        else:
                            nc.vector.tensor_copy(dst, src)

                    # ---------- (b): logits[p, q, n, j] += sum_l x * v ----------
                    # q slabs are 1KB = half a 2KB PSUM bank; start=True clears the
                    # whole bank, so only even q (bank-leading slab) starts at t=0
                    for q in range(Q):
                        nc.tensor.matmul(
                            logits[:, q].rearrange("p n j -> p (n j)"),
                            xT_sb[:, q, :],
                            vblk[:],
                            start=(t == 0 and q % 2 == 0),
                            stop=(t == ITERS - 2 and q == Q - 1),
                            skip_group_check=True,
                        )

                # ---------- output ----------
                nc.sync.dma_start(out=o_d[:].transpose([1, 0, 2]), in_=oc)

    nc.finalize()
    return nc


def kernel(x, weight):
    global LAST_RESULT
    from concourse.bass_utils import run_bass_kernel_spmd

    if "nc" not in _cache:
        _cache["nc"] = _build()
    nc = _cache["nc"]

    x = np.ascontiguousarray(np.asarray(x, dtype=np.float32))
    weight = np.ascontiguousarray(np.asarray(weight, dtype=np.float32))

    in_maps = [
        {"x": x[c * NPC : (c + 1) * NPC], "weight": weight} for c in range(NCORES)
    ]
    last_exc = None
    for attempt in range(3):
        try:
            res = run_bass_kernel_spmd(nc, in_maps, core_ids=list(range(NCORES)))
            break
        except Exception as e:
            last_exc = e
            import time

            time.sleep(5 * (attempt + 1))
    else:
        raise last_exc
    LAST_RESULT = res
    return np.concatenate([r["out"] for r in res.results], axis=0)


# revision 27
# speedup vs baseline: 1.0929x; 1.0929x over previous
"""CapsuleLinear dynamic-routing kernel for TRN2 (8 NeuronCores, data-parallel over batch).

Math (reference):
    priors[n,j,i,k] = sum_l x[n,i,l] * w[j,k,l]          (never materialized: 151MB)
    3 routing iterations entirely in the L=8 compressed space:
      probs = softmax_j(logits)                          logits[n,i,j], init 0
      s[n,j,l]  = sum_i probs[n,j,i] * x[n,i,l]          (PE matmul, contraction over i)
      u[n,j,k]  = sum_l w[j,k,l] * s[n,j,l]              (DVE broadcast-mul + reduce)
      out       = squash_k(u)
      v[n,j,l]  = sum_k w[j,k,l] * out[n,j,k]            (DVE)
      logits   += sum_l x[n,i,l] * v[n,j,l]              (PE matmul, PSUM-resident accum)

Layout: i = 9*p + q  (p = SBUF partition 0..127, q = 0..8).
Softmax normalization is folded into x (xs = x * 1/d per i) so the wide (64-per-i)
probs tensor is never divided; sqrt is computed as exp(0.5*ln(.)) and Exp/Ln are
pinned to the single natural_log_exp ACT table set (no mid-kernel table reloads).
Matmul operands are bf16 (KFP32=1 env reverts to f32); all PSUM accumulation is f32.
"""

import os

import numpy as np

N, I, L, J, K = 32, 1152, 8, 64, 16
NCORES = 8
NPC = N // NCORES  # samples per core = 4
P = 128
Q = I // P  # 9
ITERS = 3
EPS = 1e-9

_cache = {}
LAST_RESULT = None


def _patch_act_tables():
    """Restrict Exp/Ln to the one table set containing both, so bacc's
    table-load pass never alternates sets (each reload costs ~2.7us)."""
    import concourse.hw_specs as hw_specs
    from concourse import mybir

    import concourse.bacc as bacc

    if getattr(hw_specs, "_capsule_patched", False):
        return
    orig = hw_specs.get_activation_tables

    def patched(arch):
        t = dict(orig(arch))
        AF = mybir.ActivationFunctionType
        both = "natural_log_exp_and_others"
        if both in t:
            for name in t:
                if name != both:
                    t[name] = t[name] - {AF.Exp, AF.Ln}
        return t

    hw_specs.get_activation_tables = patched
    bacc.get_activation_tables = patched  # bacc binds the name via from-import
    hw_specs._capsule_patched = True


def _build():
    import concourse.bacc as bacc
    import concourse.tile as tile
    from concourse import mybir
    from concourse.masks import make_identity

    _patch_act_tables()

    f32 = mybir.dt.float32
    bf16 = mybir.dt.bfloat16
    mode = os.environ.get("KPREC", "f32")  # split | split2 | bf16 | f32
    mmdt_a = bf16 if mode in ("split", "bf16") else f32   # (a)-path: e, xs, ones
    mmdt_b = bf16 if mode in ("split2", "bf16") else f32  # (b)-path: xT, vblk, identity
    AF = mybir.ActivationFunctionType

    nc = bacc.Bacc("TRN2", target_bir_lowering=False, debug=False, num_devices=NCORES)

    x_d = nc.dram_tensor("x", (NPC, I, L), f32, kind="ExternalInput")
    w_d = nc.dram_tensor("weight", (J, K, L), f32, kind="ExternalInput")
    o_d = nc.dram_tensor("out", (NPC, J, K), f32, kind="ExternalOutput")

    with tile.TileContext(nc) as tc:
        with tc.tile_pool(name="singles", bufs=1) as singles, \
             tc.tile_pool(name="work", bufs=2) as work:
            # ---- persistent SBUF tensors ----
            # xall[p, n, q, l] = x[n, 9p+q, l]; 288B contiguous runs; split in two
            # DMAs (different queues) to halve load latency
            xall = singles.tile([P, NPC, Q, L], f32)
            xsrc = x_d[:].rearrange("n (p q) l -> p n q l", p=P)
            nc.sync.dma_start(out=xall[:, 0:2], in_=xsrc[:, 0:2])
            nc.scalar.dma_start(out=xall[:, 2:4], in_=xsrc[:, 2:4])
            w_sb = singles.tile([J, K, L], f32)
            nc.sync.dma_start(out=w_sb, in_=w_d[:])

            id_t = singles.tile([P, P], mmdt_b)
            make_identity(nc, id_t)
            ones_t = singles.tile([P, P], mmdt_a)
            nc.gpsimd.memset(ones_t, 1.0)
            # block-diag v, padded: sample n occupies partitions 32n..32n+8
            # (compute-engine APs need 32-aligned base partitions)
            vblk = singles.tile([P, NPC * J], mmdt_b)
            nc.gpsimd.memset(vblk, 0.0)
            eps_t = singles.tile([J, 1], f32)
            nc.gpsimd.memset(eps_t, EPS)
            # padded v (pre-transpose): sample n in cols 32n..32n+8, zeros between
            vT_pad = singles.tile([J, NPC, 32], mmdt_b)
            nc.gpsimd.memset(vT_pad, 0.0)

            # warm the ACT ln/exp table set while DMAs run
            warm = singles.tile([1, 2], f32)
            nc.scalar.activation(warm[:, 0:1], eps_t[0:1, 0:1], AF.Ln)
            nc.scalar.activation(warm[:, 1:2], eps_t[0:1, 0:1], AF.Exp)

            # xqnl[p, q, n, l]: contiguous (n,l) runs per q -> single-free-dim
            # matmul operands (PE moving APs allow only one free dimension)
            xqnl = singles.tile([P, Q, NPC, L], f32)
            nc.vector.tensor_copy(xqnl, xall[:].transpose([0, 2, 1, 3]))
            # padded + cast copy: sample n in cols 32n..32n+8 per q, so one PE
            # transpose per q lands sample n at partitions 32n..32n+8
            xpad = singles.tile([P, Q, NPC, 32], mmdt_b)
            nc.gpsimd.memset(xpad, 0.0)
            nc.gpsimd.tensor_copy(xpad[:, :, :, 0:L], xqnl[:])

            # ---- setup: xT[32n+l, q, p] = x[n, 9p+q, l] via 9 PE transposes ----
            xT_sb = singles.tile([P, Q, P], mmdt_b)
            with tc.tile_pool(name="setup_ps", bufs=2, space="PSUM") as setup_ps:
                for q in range(Q):
                    xT_ps = setup_ps.tile([P, P], mmdt_b)
                    nc.tensor.transpose(
                        xT_ps, xpad[:, q].rearrange("p n l -> p (n l)"), id_t
                    )
                    if q % 2 == 0:
                        nc.scalar.copy(xT_sb[:, q, :], xT_ps)
                    else:
                        nc.vector.tensor_copy(xT_sb[:, q, :], xT_ps)

            with tc.tile_pool(name="logits_ps", bufs=1, space="PSUM") as logits_pool, \
                 tc.tile_pool(name="s2_ps", bufs=1, space="PSUM") as s2_pool, \
                 tc.tile_pool(name="vtr_ps", bufs=1, space="PSUM") as vtr_pool:
                logits = logits_pool.tile([P, Q, NPC, J], f32)  # PSUM-resident

                oc = None
                for t in range(ITERS):
                    # ---------- probs folded into xs = x/sum_j exp(logits) ----------
                    # slab-grouped (3 q's at a time) so (a) matmuls for early slabs
                    # overlap softmax work on later slabs
                    xs = work.tile([P, Q, NPC, L], mmdt_a, tag="xs")
                    if t == 0:
                        nc.vector.tensor_scalar_mul(xs, xqnl[:], 1.0 / J)
                        e_t = None
                    else:
                        e_t = work.tile([P, Q, NPC, J], mmdt_a, tag="e")
                        d_t = work.tile([P, Q, NPC], f32, tag="d")
                        r_t = work.tile([P, Q, NPC], f32, tag="r")
                        for g in range(3):
                            sl = slice(3 * g, 3 * g + 3)
                            nc.scalar.activation(e_t[:, sl], logits[:, sl], AF.Exp)
                            nc.vector.tensor_reduce(
                                d_t[:, sl],
                                e_t[:, sl],
                                axis=mybir.AxisListType.X,
                                op=mybir.AluOpType.add,
                            )
                            nc.vector.reciprocal(r_t[:, sl], d_t[:, sl])
                            nc.gpsimd.tensor_mul(
                                xs[:, sl],
                                xqnl[:, sl],
                                r_t[:, sl].unsqueeze(3).broadcast_to((P, 3, NPC, L)),
                            )

                    # ---------- (a): s2[(nh,j),(nh',l)] = sum_i e * xs ----------
                    # separate PSUM tiles per half: start=True clears a whole 2KB
                    # bank, so the two halves must not share one
                    s2a = s2_pool.tile([P, 2 * L], f32, tag="s2a")
                    s2b = s2_pool.tile([P, 2 * L], f32, tag="s2b")
                    for q in range(Q):
                        for h, s2 in enumerate((s2a, s2b)):
                            lhsT = (
                                ones_t[:]
                                if t == 0
                                else e_t[:, q, 2 * h : 2 * h + 2, :].rearrange(
                                    "p n j -> p (n j)"
                                )
                            )
                            nc.tensor.matmul(
                                s2[:],
                                lhsT,
                                xs[:, q, 2 * h : 2 * h + 2, :].rearrange(
                                    "p n l -> p (n l)"
                                ),
                                start=(q == 0),
                                stop=(q == Q - 1),
                            )
                    # extract diag blocks -> s_sb[j, n, l]
                    s_sb = work.tile([J, NPC, L], f32, tag="s_sb")
                    for n in range(NPC):
                        h, n2 = divmod(n, 2)
                        s2 = s2a if h == 0 else s2b
                        src = s2[64 * n2 : 64 * n2 + 64, 8 * n2 : 8 * n2 + 8]
                        if n % 2 == 0:
                            nc.scalar.copy(s_sb[:, n, :], src)
                        else:
                            nc.vector.tensor_copy(s_sb[:, n, :], src)

                    # ---------- u[j, n, k] = sum_l w[j,k,l] * s[j,n,l] ----------
                    pt = work.tile([J, NPC, K, L], f32, tag="pt")
                    nc.vector.tensor_mul(
                        pt,
                        w_sb[:].unsqueeze(1).broadcast_to((J, NPC, K, L)),
                        s_sb[:].unsqueeze(2).broadcast_to((J, NPC, K, L)),
                    )
                    u_t = work.tile([J, NPC, K], f32, tag="u")
                    nc.vector.reduce_sum(u_t, pt, axis=mybir.AxisListType.X)

                    # ---------- squash ----------
                    u2 = work.tile([J, NPC, K], f32, tag="u2")
                    nc.gpsimd.tensor_mul(u2, u_t, u_t)
                    sq = work.tile([J, NPC], f32, tag="sq")
                    nc.vector.reduce_sum(sq, u2, axis=mybir.AxisListType.X)
                    rt = work.tile([J, NPC], f32, tag="rt")  # sqrt(sq+eps)
                    nc.scalar.activation(rt, sq, AF.Ln, bias=eps_t[:])
                    nc.scalar.activation(rt, rt, AF.Exp, scale=0.5)
                    sp1 = work.tile([J, NPC], f32, tag="sp1")  # (1+sq)*sqrt(sq+eps)
                    nc.vector.tensor_scalar_add(sp1, sq, 1.0)
                    nc.vector.tensor_mul(sp1, sp1, rt)
                    nc.vector.reciprocal(sp1, sp1)
                    scl = work.tile([J, NPC], f32, tag="scl")
                    nc.vector.tensor_mul(scl, sq, sp1)
                    oc = work.tile([J, NPC, K], f32, tag="oc")
                    nc.vector.tensor_mul(
                        oc, u_t, scl[:].unsqueeze(2).broadcast_to((J, NPC, K))
                    )

                    if t == ITERS - 1:
                        break

                    # ---------- v[j, n, l] = sum_k w[j,k,l] * oc[j,n,k] ----------
                    qt = work.tile([J, NPC, L, K], f32, tag="qt")
                    nc.vector.tensor_mul(
                        qt,
                        w_sb[:].transpose([0, 2, 1]).unsqueeze(1).broadcast_to((J, NPC, L, K)),
                        oc[:].unsqueeze(2).broadcast_to((J, NPC, L, K)),
                    )
                    vT = work.tile([J, NPC, L], f32, tag="vT")
                    nc.vector.reduce_sum(vT, qt, axis=mybir.AxisListType.X)
                    # cast into the padded layout (sample n at cols 32n..32n+8)
                    nc.scalar.copy(vT_pad[:, :, 0:L], vT[:])

                    # one PE transpose: (64, 128) -> (128, 64); sample n lands at
                    # partitions 32n..32n+8 (32-aligned, so copies below are legal)
                    vtr = vtr_pool.tile([P, J], mmdt_b, tag="vtr")
                    nc.tensor.transpose(
                        vtr, vT_pad[:].rearrange("j n l -> j (n l)"), id_t[:J, :J]
                    )
                    for n in range(NPC):
                        src = vtr[32 * n : 32 * n + 8, :]
                        dst = vblk[32 * n : 32 * n + 8, 64 * n : 64 * n + 64]
                        if n % 2 == 0:
                            nc.scalar.copy(dst, src)
                        else:
                            nc.vector.tensor_copy(dst, src)

                    # ---------- (b): logits[p, q, n, j] += sum_l x * v ----------
                    # q slabs are 1KB = half a 2KB PSUM bank; start=True clears the
                    # whole bank, so only even q (bank-leading slab) starts at t=0
                    for q in range(Q):
                        nc.tensor.matmul(
                            logits[:, q].rearrange("p n j -> p (n j)"),
                            xT_sb[:, q, :],
                            vblk[:],
                            start=(t == 0 and q % 2 == 0),
                            stop=(t == ITERS - 2 and q == Q - 1),
                            skip_group_check=True,
                        )

                # ---------- output ----------
                nc.sync.dma_start(out=o_d[:].transpose([1, 0, 2]), in_=oc)

    nc.finalize()
    return nc


def kernel(x, weight):
    global LAST_RESULT
    from concourse.bass_utils import run_bass_kernel_spmd

    if "nc" not in _cache:
        _cache["nc"] = _build()
    nc = _cache["nc"]

    x = np.ascontiguousarray(np.asarray(x, dtype=np.float32))
    weight = np.ascontiguousarray(np.asarray(weight, dtype=np.float32))

    in_maps = [
        {"x": x[c * NPC : (c + 1) * NPC], "weight": weight} for c in range(NCORES)
    ]
    last_exc = None
    for attempt in range(3):
        try:
            res = run_bass_kernel_spmd(nc, in_maps, core_ids=list(range(NCORES)))
            break
        except Exception as e:
            last_exc = e
            import time

            time.sleep(5 * (attempt + 1))
    else:
        raise last_exc
    LAST_RESULT = res
    return np.concatenate([r["out"] for r in res.results], axis=0)
